# revision 1
# baseline (speedup 1.0000x reference)
"""Multi-head graph-attention layer for Trainium2 (8-core SPMD).

The reference computes per-head projections hp = einsum("bnf,hfd->bhnd", h, W),
dense attention scores e = hp @ hp^T, LeakyReLU, softmax over the last axis,
and then multiplies hp by sum_j(softmax(e))_j. The sum of a softmax over its
own normalization axis is identically 1, so the layer's exact mathematical
output is hp itself (concatenated over heads):

    out[b, n, h*64+d] = sum_f h[b,n,f] * W[h,f,d]  =  (h[b] @ Wc)[n, h*64+d]

with Wc[f, h*64+d] = W[h,f,d]. The reference's deviation from rowsum==1 is
fp32 rounding noise (~1e-6 relative) that no reimplementation reproduces, so
computing the projection directly is both the fastest and the most accurate
realization. `adj` is unused by the reference and is ignored here.

Sharding: data-parallel over the batch dim B=8, one graph per NeuronCore.
Each core computes Y[b]^T = (Wc^T @ h[b]^T) as a [256,256] x [256,2048]
matmul with Wc chunks stationary on the PE (float32r: single-pass reduced-
precision fp32 at 1 cycle/row, measured rel err 1.4e-4 vs 2.2e-3 for bf16).

Pipeline (per core, HW-trace-tuned):
- Host packs [Wc | X^T] row-wise so each k-chunk streams as single-run
  contiguous DMAs on the sync HWDGE queue (~400 GB/s once ramped); a tiny
  dummy read pulls the queue out of its ~100 GB/s slow-start earlier.
- k-chunk input halves are interleaved so the first node-half's
  accumulation groups close while the second half streams in; their
  output DMAs overlap the input tail.
- Scratch warm-up matmuls run during the DMA wait and short filler
  matmuls plug PE idle gaps, holding the HAM clock at 2.4 GHz (cold
  matmuls are 2x slower).
- PSUM eviction alternates DVE/ACT; each engine's chunks fly out on its
  own HWDGE queue (sync issues DVE's chunks) to avoid cross-engine
  stalls and split the ~370 GB/s write bandwidth.
"""

import numpy as np

import concourse.bass as bass
import concourse.mybir as mybir
import concourse.tile as tile
from concourse import bacc
from concourse.bass_utils import run_bass_kernel_spmd

B = 8          # graphs == cores
N = 2048       # nodes per graph
F_IN = 256     # input features (= contraction dim K)
F_OUT = 256    # num_heads * d_head
P = 128        # SBUF/PSUM partitions
NTILE = 512    # PSUM bank free-dim (fp32)

KC = F_IN // P     # 2 contraction chunks
MC = F_OUT // P    # 2 output-feature chunks
NC_ = N // NTILE   # 4 node chunks
XSPLIT = 2         # node-dim halves per x DMA
XW = N // XSPLIT   # 1024

N_WARMUP_MM = 8    # scratch matmuls covering the input-DMA wait

# PE matmul dtype: float32 (exact, 4 cycles/row), float32r (reduced-precision
# single pass, 1 cycle/row at N=512, rel err ~1.4e-4), bfloat16 (1 cycle/row,
# half input DMA, rel err ~2.2e-3).
MATMUL_DTYPE = "float32r"

_module_cache = {}

# test.py reads this after calling kernel() to get profile/exec-time info.
LAST_RESULTS = None


def _build_module(mm_dtype: str) -> bass.Bass:
    if mm_dtype == "bfloat16":
        in_dt = mybir.dt.bfloat16
    elif mm_dtype == "float32r":
        in_dt = mybir.dt.float32r
    else:
        in_dt = mybir.dt.float32

    nc = bacc.Bacc(None, target_bir_lowering=False, enable_partition_id=False)
    # Host-packed input: xin[f, 0:256] = Wc[f, :], xin[f, 256:] = X^T[f, :].
    xin = nc.dram_tensor("xin", [F_IN, F_OUT + N], in_dt, kind="ExternalInput")
    yt = nc.dram_tensor("yt", [F_OUT, N], mybir.dt.float32, kind="ExternalOutput")
    XOFF = F_OUT  # x columns start here inside a packed row

    with tile.TileContext(nc) as tc:
        with (
            tc.tile_pool(name="xpool", bufs=1) as xpool,
            tc.tile_pool(name="ypool", bufs=1) as ypool,
            tc.tile_pool(name="warmpool", bufs=1) as warmpool,
            tc.tile_pool(name="pspool", bufs=1, space="PSUM") as pspool,
        ):
            # Scratch operands for PE warm-up (zeros; values are irrelevant).
            wu = warmpool.tile([P, NTILE], mybir.dt.bfloat16, name="wu", tag="wu")
            nc.gpsimd.memset(wu[:], 0.0)
            wu_mm = wu[:]

            # Per-k packed tiles [128, 2304]: cols 0:256 weights, 256: x.
            # Two single-run DMAs per k so matmuls chase the stream; all on
            # the sync HWDGE queue in need-order.
            CUT = F_OUT + XW  # per-k split: [w | x first half], then rest
            xk_sb = [
                xpool.tile([P, F_OUT + N], in_dt, name=f"xk{k}", tag=f"xk{k}")
                for k in range(KC)
            ]
            # k-interleaved input stream on one queue (splitting across
            # queues only divides the shared ~390 GB/s and slows the
            # critical chunks). Chunk sizes shrink toward the end: big
            # chunks amortize DMA overhead early, tiny last chunks keep the
            # final MM->copy->out dependency chain short.
            SPLITS = [0, XOFF + 2 * NTILE, XOFF + 3 * NTILE,
                      XOFF + 3 * NTILE + NTILE // 2, XOFF + N]
            for lo, hi in zip(SPLITS[:-2], SPLITS[1:-1]):
                nc.sync.dma_start(xk_sb[0][:, lo:hi], xin[0:P, lo:hi])
                nc.sync.dma_start(xk_sb[1][:, lo:hi], xin[P : 2 * P, lo:hi])
            lo, hi = SPLITS[-2], SPLITS[-1]
            nc.sync.dma_start(xk_sb[0][:, lo:hi], xin[0:P, lo:hi])
            nc.sync.dma_start(xk_sb[1][:, lo:hi], xin[P : 2 * P, lo:hi])


            ps = [
                [
                    pspool.tile(
                        [P, NTILE], mybir.dt.float32, name=f"ps{m}_{n}", tag=f"ps{m}_{n}"
                    )
                    for n in range(NC_)
                ]
                for m in range(MC)
            ]
            y_sb = [
                ypool.tile([P, N], mybir.dt.float32, name=f"y{m}", tag=f"y{m}")
                for m in range(MC)
            ]

            # PE clock warm-up on scratch data while the x DMAs are in
            # flight. Runs on ps[0][0] before its real accumulation group;
            # Tile's WAW tracking keeps program order.
            for _ in range(N_WARMUP_MM):
                nc.tensor.matmul(ps[0][0][:], wu_mm[:, :P], wu_mm, start=True, stop=True)

            def filler(target, count):
                # Keep the PE activity window busy during input-wait gaps so
                # the HAM clock stays at 2.4 GHz. Targets a PSUM bank that is
                # either already evicted or about to be reset by start=True -
                # never one mid-accumulation.
                for _ in range(count):
                    nc.tensor.matmul(target[:], wu_mm[:, :P], wu_mm, start=True, stop=True)

            # Segment outer, then k: each segment's groups close right
            # after its k=1 chunk lands, so outputs fly while later
            # segments are still streaming in.
            SEGS = [[0, 1], [2], [3]]
            for si, seg in enumerate(SEGS):
                for k in range(KC):
                  # ps[1][3] is untouched until the last segment resets it;
                  # ps[0][0] is the first bank evicted, done long before.
                  filler(ps[0][0] if si == len(SEGS) - 1 else ps[1][3], 2)
                  for n in seg:
                    for m in range(MC):
                        nc.tensor.matmul(
                            ps[m][n][:],
                            xk_sb[k][:, m * P : (m + 1) * P],
                            xk_sb[k][:, XOFF + n * NTILE : XOFF + (n + 1) * NTILE],
                            start=(k == 0),
                            stop=(k == KC - 1),
                        )
                        if k == KC - 1:
                            # Eviction alternates DVE/ACT; all outputs go on
                            # the single warmed-up HWDGE stream.
                            dst = y_sb[m][:, n * NTILE : (n + 1) * NTILE]
                            yslice = yt[m * P : (m + 1) * P, n * NTILE : (n + 1) * NTILE]
                            if (2 * n + m) % 2 == 0:
                                # DVE evicts; the idle sync engine issues the
                                # store on its (ramped) queue.
                                nc.vector.tensor_copy(dst, ps[m][n][:])
                                nc.sync.dma_start(yslice, dst)
                            else:
                                # ACT evicts and issues its own store on the
                                # scalar queue - no cross-engine stall.
                                nc.scalar.copy(dst, ps[m][n][:])
                                nc.scalar.dma_start(yslice, dst)
    nc.compile()
    return nc


def _get_module() -> bass.Bass:
    if MATMUL_DTYPE not in _module_cache:
        _module_cache[MATMUL_DTYPE] = _build_module(MATMUL_DTYPE)
    return _module_cache[MATMUL_DTYPE]


def kernel(h: np.ndarray, adj: np.ndarray, W: np.ndarray, **_unused) -> np.ndarray:
    global LAST_RESULTS
    h = np.asarray(h, dtype=np.float32)
    W = np.asarray(W, dtype=np.float32)
    # Wc[f, head*64+d] = W[head, f, d]
    wc = np.ascontiguousarray(W.transpose(1, 0, 2).reshape(F_IN, F_OUT))

    if MATMUL_DTYPE == "bfloat16":
        import ml_dtypes

        cast = lambda a: np.ascontiguousarray(a.astype(ml_dtypes.bfloat16))
    else:
        cast = np.ascontiguousarray

    wc_in = cast(wc)
    in_maps = [
        {"xin": np.ascontiguousarray(np.hstack([wc_in, cast(h[b].T)]))}
        for b in range(B)
    ]
    nc = _get_module()
    res = run_bass_kernel_spmd(nc, in_maps, core_ids=list(range(B)))
    LAST_RESULTS = res

    out = np.empty((B, N, F_OUT), dtype=np.float32)
    for b in range(B):
        out[b] = res.results[b]["yt"].T
    return out



# revision 3
# speedup vs baseline: 1.2512x; 1.2512x over previous
"""Multi-head graph-attention layer for Trainium2 (8-core SPMD).

The reference computes per-head projections hp = einsum("bnf,hfd->bhnd", h, W),
dense attention scores e = hp @ hp^T, LeakyReLU, softmax over the last axis,
and then multiplies hp by sum_j(softmax(e))_j. The sum of a softmax over its
own normalization axis is identically 1, so the layer's exact mathematical
output is hp itself (concatenated over heads):

    out[b, n, h*64+d] = sum_f h[b,n,f] * W[h,f,d]  =  (h[b] @ Wc)[n, h*64+d]

with Wc[f, h*64+d] = W[h,f,d]. `adj` is unused by the reference and ignored.

Sharding: data-parallel over the batch dim B=8, one graph per NeuronCore.
Each core computes Y[b]^T = (Wc^T @ h[b]^T) as a [256,256] x [256,2048]
matmul in bf16 (measured rel err ~2.2e-3, vs the 2e-2 gate), which halves
both the input and the output DMA bytes vs fp32 — the kernel is DMA-bound
(aggregate DMA ceiling ~400 GB/s/core; total traffic 2.2 MB bf16).

Pipeline (per core, HW-trace-tuned):
- Host packs one [128, 4608] bf16 input: cols [0:512) hold Wc's two
  128-row k-chunks; each node-range r holds its Xk0|Xk1 column blocks
  side by side, so ONE dma_start per range delivers everything its
  matmul group needs (dma_start costs ~600ns of sequencer time each,
  so fewer+bigger chunks matter as much as bytes).
- All input DMAs then all output DMAs are issued on the sync engine's
  HWDGE queue: the queue is FIFO, so input packets keep strict bus
  priority and output packets start the moment the input stream drains.
- PSUM eviction converts fp32->bf16 on the fly: DVE evicts the m0 half,
  ACT evicts the m1 half in parallel, into a shared per-range SBUF tile
  that one dma_start writes out ([m0|m1] packed; host unpacks).
- Ranges are sized 256/512/512/512/256: small first range starts the PE
  ~0.4us earlier (head latency is issue+DGE+sem = ~2.2us fixed), small
  last range shortens the mm->evict->issue->transfer tail.
- Scratch warm-up matmuls run during the input-DMA wait and filler
  matmuls plug PE idle gaps (PE reaches 2.4 GHz only after ~3us of
  continuous activity; cold matmuls are 2-4x slower).
"""

import numpy as np

import concourse.bass as bass
import concourse.mybir as mybir
import concourse.tile as tile
from concourse import bacc
from concourse.bass_utils import run_bass_kernel_spmd

B = 8          # graphs == cores
N = 2048       # nodes per graph
F_IN = 256     # input features (= contraction dim K)
F_OUT = 256    # num_heads * d_head
P = 128        # SBUF/PSUM partitions
KC = 2         # contraction chunks (256 = 2*128)
MC = 2         # output-feature chunks (256 = 2*128)

# Node-range widths: small head range (earlier PE start), small tail range
# (shorter mm->evict->dma tail). Each range is one input DMA, one matmul
# group per (k,m), one output DMA.
RANGES = [256, 512, 512, 512, 256]
C0S = [0, 256, 768, 1280, 1792]          # node offsets (cumsum)
WCOLS = 2 * F_IN // P * P                # 512 cols holding Wc k0|k1
XIN_COLS = WCOLS + 2 * N                 # 4608
YT_COLS = 2 * N                          # 4096 ([m0|m1] per range)

N_WARMUP_MM = 6    # scratch matmuls covering the first input-DMA wait
# filler matmuls (free-dim 512) inserted before each range's real matmuls
FILLERS = [0, 1, 1, 1, 0]

_module_cache = {}

# test.py reads this after calling kernel() to get profile/exec-time info.
LAST_RESULTS = None


def _build_module() -> bass.Bass:
    in_dt = mybir.dt.bfloat16

    nc = bacc.Bacc(None, target_bir_lowering=False, enable_partition_id=False)
    xin = nc.dram_tensor("xin", [P, XIN_COLS], in_dt, kind="ExternalInput")
    yt = nc.dram_tensor("yt", [P, YT_COLS], mybir.dt.bfloat16, kind="ExternalOutput")

    with tile.TileContext(nc) as tc:
        with (
            tc.tile_pool(name="xpool", bufs=1) as xpool,
            tc.tile_pool(name="ypool", bufs=1) as ypool,
            tc.tile_pool(name="warmpool", bufs=1) as warmpool,
            tc.tile_pool(name="pspool", bufs=1, space="PSUM") as pspool,
        ):
            # Scratch operands for PE warm-up (zeros; values are irrelevant).
            wu = warmpool.tile([P, 512], mybir.dt.bfloat16, name="wu", tag="wu")
            nc.gpsimd.memset(wu[:], 0.0)
            wu_mm = wu[:]

            x_sb = xpool.tile([P, XIN_COLS], in_dt, name="x", tag="x")
            y_sb = [
                ypool.tile([P, 2 * w], mybir.dt.bfloat16, name=f"y{r}", tag=f"y{r}")
                for r, w in enumerate(RANGES)
            ]

            # Input DMAs in need-order on the sync HWDGE queue. Chunk 0
            # carries Wc plus range 0; chunk r carries range r's Xk0|Xk1.
            bounds = [0]
            for r, w in enumerate(RANGES):
                start = WCOLS + 2 * C0S[r]
                bounds.append(start + 2 * w)
            for lo, hi in zip(bounds[:-1], bounds[1:]):
                nc.sync.dma_start(x_sb[:, lo:hi], xin[:, lo:hi])

            # PSUM: ranges 1-3 own a bank per m; ranges 0 and 4 (256 cols
            # each) share bank 0 of each m. 2m x 4 banks = all 8 banks.
            ps = [
                [
                    pspool.tile([P, 512], mybir.dt.float32, name=f"ps{m}_{j}", tag=f"ps{m}_{j}")
                    for j in range(4)
                ]
                for m in range(MC)
            ]

            def ps_slice(m, r):
                if r == 0:
                    return ps[m][0][:, 0:256]
                if r == 4:
                    return ps[m][0][:, 256:512]
                return ps[m][r][:]

            # PE clock warm-up on scratch data while the first chunk is in
            # flight (targets ps[1][3]: its real group starts last among
            # banks 1-3; Tile's WAW tracking keeps program order).
            for _ in range(N_WARMUP_MM):
                nc.tensor.matmul(ps[1][3][:], wu_mm[:, :P], wu_mm, start=True, stop=True)

            def filler(r, count):
                # Keep the PE busy during input-wait gaps so the clock stays
                # ramped. Target a bank that is not mid-accumulation: ps[1][3]
                # until range 3 starts, then range 0's (already evicted) region.
                tgt = ps[1][3][:] if r < 3 else ps[0][0][:, 0:256]
                rhs = wu_mm if r < 3 else wu_mm[:, :256]
                for _ in range(count):
                    nc.tensor.matmul(tgt, wu_mm[:, :P], rhs, start=True, stop=True)

            for r, w in enumerate(RANGES):
                c0 = C0S[r]
                xoff = WCOLS + 2 * c0
                filler(r, FILLERS[r])
                for k in range(KC):
                    for m in range(MC):
                        nc.tensor.matmul(
                            ps_slice(m, r),
                            x_sb[:, k * F_OUT + m * P : k * F_OUT + (m + 1) * P],
                            x_sb[:, xoff + k * w : xoff + (k + 1) * w],
                            start=(k == 0),
                            stop=(k == KC - 1),
                        )
                # Evict fp32 PSUM -> bf16 SBUF: DVE does m0, ACT does m1,
                # in parallel; then one output DMA for the packed [m0|m1]
                # tile, queued on sync's (input-priority FIFO) queue.
                nc.vector.tensor_copy(y_sb[r][:, 0:w], ps_slice(0, r))
                nc.scalar.copy(y_sb[r][:, w : 2 * w], ps_slice(1, r))
                nc.sync.dma_start(yt[:, 2 * c0 : 2 * c0 + 2 * w], y_sb[r][:])
    nc.compile()
    return nc


def _get_module() -> bass.Bass:
    if "m" not in _module_cache:
        _module_cache["m"] = _build_module()
    return _module_cache["m"]


def kernel(h: np.ndarray, adj: np.ndarray, W: np.ndarray, **_unused) -> np.ndarray:
    global LAST_RESULTS
    import ml_dtypes

    bf16 = ml_dtypes.bfloat16
    h = np.asarray(h, dtype=np.float32)
    W = np.asarray(W, dtype=np.float32)
    # Wc[f, head*64+d] = W[head, f, d]
    wc = np.ascontiguousarray(W.transpose(1, 0, 2).reshape(F_IN, F_OUT)).astype(bf16)

    in_maps = []
    for b in range(B):
        xt = h[b].T.astype(bf16)  # [256 f, 2048 n]
        xin = np.empty((P, XIN_COLS), dtype=bf16)
        xin[:, 0:F_OUT] = wc[0:P]
        xin[:, F_OUT : 2 * F_OUT] = wc[P : 2 * P]
        for r, w in enumerate(RANGES):
            c0 = C0S[r]
            s = WCOLS + 2 * c0
            xin[:, s : s + w] = xt[0:P, c0 : c0 + w]
            xin[:, s + w : s + 2 * w] = xt[P : 2 * P, c0 : c0 + w]
        in_maps.append({"xin": xin})

    nc = _get_module()
    res = run_bass_kernel_spmd(nc, in_maps, core_ids=list(range(B)))
    LAST_RESULTS = res

    out = np.empty((B, N, F_OUT), dtype=np.float32)
    yt_full = np.empty((F_OUT, N), dtype=np.float32)
    for b in range(B):
        ytb = res.results[b]["yt"]
        for r, w in enumerate(RANGES):
            c0 = C0S[r]
            blk = ytb[:, 2 * c0 : 2 * c0 + 2 * w].astype(np.float32)
            yt_full[0:P, c0 : c0 + w] = blk[:, 0:w]
            yt_full[P : 2 * P, c0 : c0 + w] = blk[:, w : 2 * w]
        out[b] = yt_full.T
    return out
